# revision 14
# baseline (speedup 1.0000x reference)
"""Trainium2 Bass kernel for nn_Correlation (max_disp=4).

out[b, k, h, w] = mean_c x1[b,c,h,w] * pad(x2)[b,c,h+dx,w+dy],
k = 9*dx + dy, dx,dy in [0,9), pad = 4 zeros on each spatial side.

Strategy (batch-parallel over 8 cores, one batch sample per core):

The correlation needs the 9 diagonals g[w, w+dy] of each per-(h,dx) Gram
matrix g[w, u] = sum_c x1[c,h,w] * x2p[c,h+dx,u].  Per-partition (per-w)
column offsets are unaddressable on-chip, so instead of computing the
full [128, 136] gram and extracting diagonals through a DRAM bounce
(descriptor-bound: 81 * 18B scattered reads per (h,w)), the PE array
itself shears the gram at 32-column granularity:

  - Per output row h, FOUR column-tiled matmuls (tile_position=(0,32q))
    run concurrently on the 128x128 PE array.  Tile q covers output
    partitions w in [32q, 32q+32) and streams a 40-column x2 window:
    ps[w, 9*dx + t] = sum_c x1[c,h,w] * x2[c, h+dx-4, 32q+t-4].
    All 81 correlation values for pixel (h,w) live at t = (w%32) + dy.
  - No zero-padding on device: out-of-range x2 indices read garbage,
    but every (k,h,w) whose shifted index is out of range is exactly 0
    in the reference (the pad zeroes the whole product), so the host
    zeroes those border outputs during extraction.  Out-of-range h rows
    are skipped by narrowing the matmul (stale PSUM there is likewise
    discarded by the host).
  - ScalarE/VectorE (alternating per h) drain PSUM -> SBUF f16; blocks
    of 8 rows stream to HBM as contiguous 0.74 MB DMAs.
  - Host: t = (w%32)+dy take_along_axis + border zeroing + 1/128 scale.

Inputs are pre-cast to f16 on the host (halves read traffic; loads use
plain HWDGE on both rings).  HBM traffic per core: 8 MB in + 11.8 MB
out, ~4.6K DMA packets, all contiguous >=4 KB per partition.
"""

import sys

if "/opt/trn_rl_repo" not in sys.path:
    sys.path.insert(0, "/opt/trn_rl_repo")

import numpy as np

B, C, H, W = 8, 128, 128, 128
D = 4
ND = 2 * D + 1  # 9
NK = ND * ND  # 81
TW = 40  # t-window per 32-col group: (w%32) + dy < 32 + 9
GB = ND * TW  # 360 = per-h band row
HB = 16  # h rows per output staging tile (stored as two 8-row DMAs)
SB = 8  # h rows per store DMA
NCHUNK = 4  # input load chunks (32 rows each, 1 MB)
PADL = 4  # leading guard elements in the flat x2 tile

_cache = {}


def _build():
    from contextlib import ExitStack

    import concourse.mybir as mybir
    from concourse.ap import AP
    from concourse import bacc
    from concourse.bass import MemorySpace
    from concourse.tile import TileContext

    f16 = mybir.dt.float16
    f32 = mybir.dt.float32

    nc = bacc.Bacc("TRN2", target_bir_lowering=False, debug=False)
    X1 = nc.declare_dram_parameter("x1", [C, H, W], f16, isOutput=False)
    X2 = nc.declare_dram_parameter("x2", [C, H, W], f16, isOutput=False)
    ST = nc.declare_dram_parameter("st", [W, H, GB], f16, isOutput=True)

    HW8 = H * W + 2 * PADL  # flat x2 row + guard on both ends

    with TileContext(nc) as tc, ExitStack() as ctx:
        consts = ctx.enter_context(tc.tile_pool(name="consts", bufs=1))
        sts = ctx.enter_context(tc.tile_pool(name="sts", bufs=4))
        psums = ctx.enter_context(
            tc.tile_pool(name="psums", bufs=8, space=MemorySpace.PSUM)
        )

        x1_sb = consts.tile([C, H, W], f16)
        x2_sb = consts.tile([C, HW8], f16)  # rows at offset PADL, flat

        hc = H // NCHUNK
        for r in range(NCHUNK):
            # x1 chunks on the SP HWDGE ring, x2 chunks on the ACT ring
            nc.sync.dma_start(
                x1_sb[:, r * hc : (r + 1) * hc, :], X1[:, r * hc : (r + 1) * hc, :]
            )
            nc.scalar.dma_start(
                x2_sb[:, PADL + r * hc * W : PADL + (r + 1) * hc * W],
                X2[:, r * hc : (r + 1) * hc, :],
            )

        x2_full = x2_sb[:]
        x2_part_dim = list(x2_full.ap)[0]  # [partition pitch, 128]

        st_sb = None
        for h in range(H):
            hl = h % HB
            if hl == 0:
                st_sb = sts.tile([W, HB, GB], f16)
            ps = psums.tile([W, GB], f32)
            dx0 = max(0, 4 - h)  # first in-range dx block
            hlo = h - 4 + dx0
            nrows = min(H, h + 5) - hlo
            for q in range(4):
                rhs = AP(
                    x2_full.tensor,
                    x2_full.offset + PADL + hlo * W + 32 * q - 4,
                    [x2_part_dim, [W, nrows], [1, TW]],
                )
                nc.tensor.matmul(
                    ps[32 * q : 32 * (q + 1), dx0 * TW : (dx0 + nrows) * TW],
                    x1_sb[:, h, 32 * q : 32 * q + 32],
                    rhs,
                    start=True,
                    stop=True,
                    tile_position=(0, 32 * q),
                )
            # split drains evenly between the two PSUM-capable engines
            if h % 2:
                nc.vector.tensor_copy(st_sb[:, hl, :], ps[:])
            else:
                nc.scalar.copy(st_sb[:, hl, :], ps[:])
            # store in 8-row half-blocks, alternating the two HWDGE rings
            if h % SB == SB - 1:
                s0 = (h % HB) - SB + 1
                eng = nc.sync if (h // SB) % 2 == 0 else nc.scalar
                eng.dma_start(
                    ST[:, h - SB + 1 : h + 1, :], st_sb[:, s0 : s0 + SB, :]
                )

    nc.finalize()
    return nc


def _get_program():
    if "prog" not in _cache:
        _cache["prog"] = _build()
    return _cache["prog"]


# host-side extraction indices: t = (w % 32) + dy
_T_IDX = (np.arange(W) % 32)[:, None] + np.arange(ND)[None, :]  # [W, ND]
# border masks: zero wherever the shifted x2 index is out of range
_SH = np.arange(ND)[:, None] + np.arange(H)[None, :] - D  # [dx, h] -> h+dx-4
_MH = ((_SH >= 0) & (_SH < H)).astype(np.float32)  # [dx, h]
_MW = ((_SH >= 0) & (_SH < W)).astype(np.float32)  # [dy, w]
_MASK = (_MH[:, None, :, None] * _MW[None, :, None, :]).reshape(NK, H, W)


def _extract(st_all):
    """st_all: [B, W, H, ND*TW] f16 -> out [B, NK, H, W] f32."""
    st = st_all.reshape(B, W, H, ND, TW)
    idx = np.broadcast_to(_T_IDX[None, :, None, None, :], (B, W, H, ND, ND))
    g = np.take_along_axis(st, idx, axis=4)  # [B, w, h, dx, dy]
    out = g.transpose(0, 3, 4, 2, 1).astype(np.float32).reshape(B, NK, H, W)
    # np.where (not *=): garbage reads can be NaN/Inf and 0*NaN = NaN
    out = np.where(_MASK[None] != 0, out, np.float32(0.0))
    out *= np.float32(1.0 / C)
    return np.ascontiguousarray(out)


def _run(x_1, x_2, trace=False):
    from concourse.bass_utils import run_bass_kernel_spmd

    nc = _get_program()
    x_1 = np.ascontiguousarray(np.asarray(x_1, dtype=np.float16))
    x_2 = np.ascontiguousarray(np.asarray(x_2, dtype=np.float16))
    in_maps = [{"x1": x_1[i], "x2": x_2[i]} for i in range(B)]
    res = run_bass_kernel_spmd(nc, in_maps, core_ids=list(range(B)), trace=trace)
    st_all = np.stack([res.results[i]["st"] for i in range(B)], axis=0)
    return _extract(st_all), res


def kernel(x_1, x_2):
    out, _ = _run(x_1, x_2)
    return out
